# revision 1
# baseline (speedup 1.0000x reference)
"""Trainium2 Bass kernel for nn_EntangledDeltaTreeModel.

Tree: branching B=8, depth D=7, LAYER=16 weights per node.
  - leaf weights = sum of deltas along root-to-leaf path
  - delta_loss  = sum over levels>=1 of sum(rowsum|d_l| / max(|h_l - h_parent|, 1e-7))
  - leaf NN: hidden = tanh(x @ W + b); out = hidden . ow + ob  (per-leaf 3x3 weights)

Sharding: 8 root-subtrees -> 8 NeuronCores (axis-0 shard of leaves/nodes).
Host computes the tiny level 0..6 prefix (0.3% of nodes) + its loss and ships
per-core: transposed SoA planes of level-7 deltas [16, 262144], x [3, 262144],
accumulated level-6 weights [16, 32768], and heights. Device does the level-7
repeat+add, the per-leaf NN, and the level-7 loss; host sums the scalar loss.
"""

import sys

sys.path.insert(0, "/opt/trn_rl_repo")

import numpy as np

B = 8
D = 7
LAYER = 16
MIN_DIST = 1e-7
NCORES = 8

_SIZES = [B**l for l in range(D + 1)]
_OFFS = np.concatenate([[0], np.cumsum(_SIZES)]).astype(int)
N_NODES = int(_OFFS[-1])
N_LEAVES = B**D

LPC = N_LEAVES // NCORES  # 262144 leaves per core
COLS = LPC // 128  # 2048 free columns per core
F = 256  # columns per chunk
NCHUNK = COLS // F  # 8
C = LAYER  # 16
N6C = LPC // B  # 32768 level-6 nodes per core
N6COLS = N6C // 128  # 256

_STATE = {}


def _build():
    import concourse.bacc as bacc
    import concourse.mybir as mybir
    from concourse.tile import TileContext

    fp32 = mybir.dt.float32
    op = mybir.AluOpType

    nc = bacc.Bacc("TRN2", target_bir_lowering=False)

    d7 = nc.dram_tensor("d7", [C, LPC], fp32, kind="ExternalInput")
    x3 = nc.dram_tensor("x3", [3, LPC], fp32, kind="ExternalInput")
    a6 = nc.dram_tensor("a6", [C, N6C], fp32, kind="ExternalInput")
    h6 = nc.dram_tensor("h6", [N6C], fp32, kind="ExternalInput")
    h7 = nc.dram_tensor("h7", [LPC], fp32, kind="ExternalInput")
    out = nc.dram_tensor("out", [128, COLS], fp32, kind="ExternalOutput")
    lp = nc.dram_tensor("lp", [128, NCHUNK], fp32, kind="ExternalOutput")

    d7r = d7[:].rearrange("c (p j) -> p c j", p=128)  # [128, 16, 2048]
    x3r = x3[:].rearrange("c (p j) -> p c j", p=128)  # [128, 3, 2048]
    a6r = a6[:].rearrange("c (p n) -> p c n", p=128)  # [128, 16, 256]
    h6r = h6[:].rearrange("(p n) -> p n", p=128)  # [128, 256]
    h7r = h7[:].rearrange("(p j) -> p j", p=128)  # [128, 2048]

    with TileContext(nc) as tc:
        with (
            tc.tile_pool(name="res", bufs=1) as res,
            tc.tile_pool(name="big", bufs=3) as big,
            tc.tile_pool(name="med", bufs=3) as med,
            tc.tile_pool(name="sml", bufs=3) as sml,
        ):
            A6 = res.tile([128, C * N6COLS], fp32)  # acc6 planes, c-major
            H6 = res.tile([128, N6COLS], fp32)
            H7 = res.tile([128, COLS], fp32)
            LP = res.tile([128, NCHUNK], fp32)
            nc.sync.dma_start(A6[:].rearrange("p (c n) -> p c n", c=C), a6r)
            nc.sync.dma_start(H6[:], h6r)
            nc.sync.dma_start(H7[:], h7r)

            for k in range(NCHUNK):
                j0 = k * F
                n0 = k * (F // B)
                TD = big.tile([128, C * F], fp32, tag="td")  # d7 chunk, c-major
                M = big.tile([128, C * F], fp32, tag="m")  # |d7|*r scratch
                TX = med.tile([128, 3 * F], fp32, tag="tx")
                PR = med.tile([128, 3 * F], fp32, tag="pr")
                PR2 = med.tile([128, 3 * F], fp32, tag="pr2")
                S = med.tile([128, 3 * F], fp32, tag="s")
                HT = med.tile([128, 3 * F], fp32, tag="ht")
                Q = med.tile([128, 3 * F], fp32, tag="q")
                O = sml.tile([128, F], fp32, tag="o")
                MUT = sml.tile([128, F], fp32, tag="mut")
                R7 = sml.tile([128, F], fp32, tag="r7")
                RS = sml.tile([128, F], fp32, tag="rs")

                nc.sync.dma_start(
                    TD[:].rearrange("p (c f) -> p c f", c=C),
                    d7r[:, :, j0 : j0 + F],
                )
                nc.sync.dma_start(
                    TX[:].rearrange("p (c f) -> p c f", c=3),
                    x3r[:, :, j0 : j0 + F],
                )

                # ---- mutation distance r7 = 1/max(|h7 - h6[parent]|, eps)
                h6v = (
                    H6[:, n0 : n0 + F // B]
                    .rearrange("p (n o) -> p n o", o=1)
                    .broadcast_to((128, F // B, B))
                )
                h7v = H7[:, j0 : j0 + F].rearrange("p (n s) -> p n s", n=F // B)
                nc.vector.tensor_tensor(
                    MUT[:].rearrange("p (n s) -> p n s", n=F // B),
                    h7v,
                    h6v,
                    op.subtract,
                )
                nc.vector.scalar_tensor_tensor(
                    MUT[:], MUT[:], -1.0, MUT[:], op.mult, op.max
                )
                nc.vector.tensor_scalar(MUT[:], MUT[:], MIN_DIST, None, op.max)
                nc.vector.reciprocal_approx_accurate(R7[:], MUT[:], RS[:])

                # ---- level-7 loss partial: sum |d7 * r7| -> LP[:, k]
                r7v = (
                    R7[:]
                    .rearrange("p (o f) -> p o f", o=1)
                    .broadcast_to((128, C, F))
                )
                nc.vector.tensor_tensor(
                    M[:].rearrange("p (c f) -> p c f", c=C),
                    TD[:].rearrange("p (c f) -> p c f", c=C),
                    r7v,
                    op.mult,
                )
                nc.scalar.activation(
                    M[:],
                    M[:],
                    mybir.ActivationFunctionType.Abs,
                    accum_out=LP[:, k : k + 1],
                )

                # ---- leaf weights: W = d7 + acc6[parent]  (in-place over TD)
                a6v = (
                    A6[:]
                    .rearrange("p (c n) -> p c n", c=C)[:, :, n0 : n0 + F // B]
                    .rearrange("p c (n o) -> p c n o", o=1)
                    .broadcast_to((128, C, F // B, B))
                )
                td4 = TD[:].rearrange("p (c n s) -> p c n s", c=C, n=F // B)
                nc.vector.tensor_tensor(td4, td4, a6v, op.add)

                # ---- hidden pre-activation: s_h = sum_d x_d * w[3d+h] + w[9+h]
                for d in range(3):
                    xv = (
                        TX[:, d * F : (d + 1) * F]
                        .rearrange("p (o f) -> p o f", o=1)
                        .broadcast_to((128, 3, F))
                    )
                    dst = (PR if d == 0 else PR2)[:].rearrange(
                        "p (h f) -> p h f", h=3
                    )
                    nc.vector.tensor_tensor(
                        dst,
                        TD[:, 3 * d * F : (3 * d + 3) * F].rearrange(
                            "p (h f) -> p h f", h=3
                        ),
                        xv,
                        op.mult,
                    )
                    if d == 1:
                        nc.vector.tensor_tensor(S[:], PR[:], PR2[:], op.add)
                    elif d == 2:
                        nc.vector.tensor_tensor(S[:], S[:], PR2[:], op.add)
                nc.vector.tensor_tensor(
                    S[:], S[:], TD[:, 9 * F : 12 * F], op.add
                )

                # ---- tanh on ScalarE
                nc.scalar.activation(
                    HT[:], S[:], mybir.ActivationFunctionType.Tanh
                )

                # ---- output: out = sum_h hidden_h * ow_h + ob  (GPSIMD)
                nc.gpsimd.tensor_tensor(
                    Q[:], HT[:], TD[:, 12 * F : 15 * F], op.mult
                )
                nc.gpsimd.tensor_tensor(
                    O[:], Q[:, 0:F], Q[:, F : 2 * F], op.add
                )
                nc.gpsimd.tensor_tensor(O[:], O[:], Q[:, 2 * F : 3 * F], op.add)
                nc.gpsimd.tensor_tensor(
                    O[:], O[:], TD[:, 15 * F : 16 * F], op.add
                )

                nc.sync.dma_start(out[:, j0 : j0 + F], O[:])

            nc.sync.dma_start(lp[:], LP[:])

    nc.compile()
    return nc


def _get_nc():
    if "nc" not in _STATE:
        _STATE["nc"] = _build()
    return _STATE["nc"]


def kernel(x, deltas, heights):
    from concourse.bass_utils import run_bass_kernel_spmd

    x = np.asarray(x, dtype=np.float32)
    deltas = np.asarray(deltas, dtype=np.float32)
    heights = np.asarray(heights, dtype=np.float32)
    o = _OFFS

    # ---- host: prefix-accumulate levels 0..6 (0.3% of nodes) + their loss
    w = deltas[0:1]
    loss_host = 0.0
    for l in range(1, D):  # levels 1..6
        d_l = deltas[o[l] : o[l + 1]]
        h_l = heights[o[l] : o[l + 1]].astype(np.float64)
        h_p = np.repeat(heights[o[l - 1] : o[l]].astype(np.float64), B)
        w = np.repeat(w, B, axis=0) + d_l
        mut = np.maximum(np.abs(h_l - h_p), MIN_DIST)
        loss_host += float(
            (np.abs(d_l.astype(np.float64)).sum(axis=1) / mut).sum()
        )
    acc6 = w  # [262144, 16] accumulated weights at level 6

    in_maps = []
    for i in range(NCORES):
        s7 = slice(o[7] + i * LPC, o[7] + (i + 1) * LPC)
        s6 = slice(o[6] + i * N6C, o[6] + (i + 1) * N6C)
        in_maps.append(
            {
                "d7": np.ascontiguousarray(deltas[s7].T),
                "x3": np.ascontiguousarray(x[i * LPC : (i + 1) * LPC].T),
                "a6": np.ascontiguousarray(acc6[i * N6C : (i + 1) * N6C].T),
                "h6": np.ascontiguousarray(heights[s6]),
                "h7": np.ascontiguousarray(heights[s7]),
            }
        )

    nc = _get_nc()
    import os

    trace = bool(int(os.environ.get("KERNEL_TRACE", "0")))
    res = run_bass_kernel_spmd(
        nc, in_maps, core_ids=list(range(NCORES)), trace=trace
    )
    _STATE["last_results"] = res

    out = np.concatenate(
        [res.results[i]["out"].reshape(-1) for i in range(NCORES)]
    )
    loss = loss_host + sum(
        float(res.results[i]["lp"].astype(np.float64).sum())
        for i in range(NCORES)
    )
    return out, np.array([loss], dtype=np.float32)


# revision 4
# speedup vs baseline: 2815.0421x; 2815.0421x over previous
"""Trainium2 Bass kernel for nn_EntangledDeltaTreeModel.

Tree: branching B=8, depth D=7, LAYER=16 weights per node.
  - leaf weights = sum of deltas along root-to-leaf path
  - delta_loss  = sum over levels>=1 of sum(rowsum|d_l| / max(|h_l - h_parent|, 1e-7))
  - leaf NN: hidden = tanh(x @ W + b); out = hidden . ow + ob  (per-leaf 3x3 weights)

Sharding: 8 root-subtrees -> 8 NeuronCores (axis-0 shard of leaves/nodes).
Host computes the tiny level 0..6 prefix (0.3% of nodes) + its loss and ships
per-core: transposed SoA planes of level-7 deltas [16, 262144], x [3, 262144],
accumulated level-6 weights [16, 32768], and heights. Device does the level-7
repeat+add, the per-leaf NN, and the level-7 loss; host sums the scalar loss.
"""

import sys

sys.path.insert(0, "/opt/trn_rl_repo")

import numpy as np

B = 8
D = 7
LAYER = 16
MIN_DIST = 1e-7
NCORES = 8

_SIZES = [B**l for l in range(D + 1)]
_OFFS = np.concatenate([[0], np.cumsum(_SIZES)]).astype(int)
N_NODES = int(_OFFS[-1])
N_LEAVES = B**D

LPC = N_LEAVES // NCORES  # 262144 leaves per core
COLS = LPC // 128  # 2048 free columns per core
F = 256  # columns per chunk
NCHUNK = COLS // F  # 8
C = LAYER  # 16
N6C = LPC // B  # 32768 level-6 nodes per core
N6COLS = N6C // 128  # 256

_STATE = {}


def _build(reps=1):
    import concourse.bacc as bacc
    import concourse.mybir as mybir
    from concourse.tile import TileContext

    fp32 = mybir.dt.float32
    op = mybir.AluOpType

    nc = bacc.Bacc("TRN2", target_bir_lowering=False)

    d7 = nc.dram_tensor("d7", [C, LPC], fp32, kind="ExternalInput")
    x3 = nc.dram_tensor("x3", [3, LPC], fp32, kind="ExternalInput")
    a6 = nc.dram_tensor("a6", [C, N6C], fp32, kind="ExternalInput")
    h6 = nc.dram_tensor("h6", [N6C], fp32, kind="ExternalInput")
    h7 = nc.dram_tensor("h7", [LPC], fp32, kind="ExternalInput")
    out = nc.dram_tensor("out", [128, COLS], fp32, kind="ExternalOutput")
    lp = nc.dram_tensor("lp", [128, NCHUNK], fp32, kind="ExternalOutput")

    d7r = d7[:].rearrange("c (p j) -> p c j", p=128)  # [128, 16, 2048]
    x3r = x3[:].rearrange("c (p j) -> p c j", p=128)  # [128, 3, 2048]
    a6r = a6[:].rearrange("c (p n) -> p c n", p=128)  # [128, 16, 256]
    h6r = h6[:].rearrange("(p n) -> p n", p=128)  # [128, 256]
    h7r = h7[:].rearrange("(p j) -> p j", p=128)  # [128, 2048]

    with TileContext(nc) as tc:
        with (
            tc.tile_pool(name="res", bufs=1) as res,
            tc.tile_pool(name="big", bufs=3) as big,
            tc.tile_pool(name="med", bufs=3) as med,
            tc.tile_pool(name="sml", bufs=3) as sml,
        ):
            A6 = res.tile([128, C * N6COLS], fp32)  # acc6 planes, c-major
            H6 = res.tile([128, N6COLS], fp32)
            H7 = res.tile([128, COLS], fp32)
            LP = res.tile([128, NCHUNK], fp32)
            nc.sync.dma_start(A6[:].rearrange("p (c n) -> p c n", c=C), a6r)
            nc.sync.dma_start(H6[:], h6r)
            nc.sync.dma_start(H7[:], h7r)

            for k in range(NCHUNK * reps):
                k = k % NCHUNK
                j0 = k * F
                n0 = k * (F // B)
                TD = big.tile([128, C * F], fp32, tag="td")  # d7 chunk, c-major
                M = big.tile([128, C * F], fp32, tag="m")  # |d7|*r scratch
                TX = med.tile([128, 3 * F], fp32, tag="tx")
                PR = med.tile([128, 3 * F], fp32, tag="pr")
                PR2 = med.tile([128, 3 * F], fp32, tag="pr2")
                S = med.tile([128, 3 * F], fp32, tag="s")
                HT = med.tile([128, 3 * F], fp32, tag="ht")
                Q = med.tile([128, 3 * F], fp32, tag="q")
                O = sml.tile([128, F], fp32, tag="o")
                MUT = sml.tile([128, F], fp32, tag="mut")
                R7 = sml.tile([128, F], fp32, tag="r7")
                RS = sml.tile([128, F], fp32, tag="rs")

                nc.sync.dma_start(
                    TD[:].rearrange("p (c f) -> p c f", c=C),
                    d7r[:, :, j0 : j0 + F],
                )
                nc.sync.dma_start(
                    TX[:].rearrange("p (c f) -> p c f", c=3),
                    x3r[:, :, j0 : j0 + F],
                )

                # ---- mutation distance r7 = 1/max(|h7 - h6[parent]|, eps)
                h6v = (
                    H6[:, n0 : n0 + F // B]
                    .rearrange("p (n o) -> p n o", o=1)
                    .broadcast_to((128, F // B, B))
                )
                h7v = H7[:, j0 : j0 + F].rearrange("p (n s) -> p n s", n=F // B)
                nc.vector.tensor_tensor(
                    MUT[:].rearrange("p (n s) -> p n s", n=F // B),
                    h7v,
                    h6v,
                    op.subtract,
                )
                nc.vector.scalar_tensor_tensor(
                    MUT[:], MUT[:], -1.0, MUT[:], op.mult, op.max
                )
                nc.vector.tensor_scalar(MUT[:], MUT[:], MIN_DIST, None, op.max)
                nc.vector.reciprocal_approx_accurate(R7[:], MUT[:], RS[:])

                # ---- level-7 loss partial: sum |d7 * r7| -> LP[:, k]
                r7v = (
                    R7[:]
                    .rearrange("p (o f) -> p o f", o=1)
                    .broadcast_to((128, C, F))
                )
                nc.vector.tensor_tensor(
                    M[:].rearrange("p (c f) -> p c f", c=C),
                    TD[:].rearrange("p (c f) -> p c f", c=C),
                    r7v,
                    op.mult,
                )
                nc.scalar.activation(
                    M[:],
                    M[:],
                    mybir.ActivationFunctionType.Abs,
                    accum_out=LP[:, k : k + 1],
                )

                # ---- leaf weights: W = d7 + acc6[parent]  (in-place over TD)
                a6v = (
                    A6[:]
                    .rearrange("p (c n) -> p c n", c=C)[:, :, n0 : n0 + F // B]
                    .rearrange("p c (n o) -> p c n o", o=1)
                    .broadcast_to((128, C, F // B, B))
                )
                td4 = TD[:].rearrange("p (c n s) -> p c n s", c=C, n=F // B)
                nc.vector.tensor_tensor(td4, td4, a6v, op.add)

                # ---- hidden pre-activation: s_h = sum_d x_d * w[3d+h] + w[9+h]
                for d in range(3):
                    xv = (
                        TX[:, d * F : (d + 1) * F]
                        .rearrange("p (o f) -> p o f", o=1)
                        .broadcast_to((128, 3, F))
                    )
                    dst = (PR if d == 0 else PR2)[:].rearrange(
                        "p (h f) -> p h f", h=3
                    )
                    nc.vector.tensor_tensor(
                        dst,
                        TD[:, 3 * d * F : (3 * d + 3) * F].rearrange(
                            "p (h f) -> p h f", h=3
                        ),
                        xv,
                        op.mult,
                    )
                    if d == 1:
                        nc.vector.tensor_tensor(S[:], PR[:], PR2[:], op.add)
                    elif d == 2:
                        nc.vector.tensor_tensor(S[:], S[:], PR2[:], op.add)
                nc.vector.tensor_tensor(
                    S[:], S[:], TD[:, 9 * F : 12 * F], op.add
                )

                # ---- tanh on ScalarE
                nc.scalar.activation(
                    HT[:], S[:], mybir.ActivationFunctionType.Tanh
                )

                # ---- output: out = sum_h hidden_h * ow_h + ob  (GPSIMD)
                nc.gpsimd.tensor_tensor(
                    Q[:], HT[:], TD[:, 12 * F : 15 * F], op.mult
                )
                nc.gpsimd.tensor_tensor(
                    O[:], Q[:, 0:F], Q[:, F : 2 * F], op.add
                )
                nc.gpsimd.tensor_tensor(O[:], O[:], Q[:, 2 * F : 3 * F], op.add)
                nc.gpsimd.tensor_tensor(
                    O[:], O[:], TD[:, 15 * F : 16 * F], op.add
                )

                nc.sync.dma_start(out[:, j0 : j0 + F], O[:])

            nc.sync.dma_start(lp[:], LP[:])

    nc.compile()
    return nc


def _get_nc(reps=1):
    key = f"nc{reps}"
    if key not in _STATE:
        _STATE[key] = _build(reps)
    return _STATE[key]


def kernel(x, deltas, heights):
    from concourse.bass_utils import run_bass_kernel_spmd

    x = np.asarray(x, dtype=np.float32)
    deltas = np.asarray(deltas, dtype=np.float32)
    heights = np.asarray(heights, dtype=np.float32)
    o = _OFFS

    # ---- host: prefix-accumulate levels 0..6 (0.3% of nodes) + their loss
    w = deltas[0:1]
    loss_host = 0.0
    for l in range(1, D):  # levels 1..6
        d_l = deltas[o[l] : o[l + 1]]
        h_l = heights[o[l] : o[l + 1]].astype(np.float64)
        h_p = np.repeat(heights[o[l - 1] : o[l]].astype(np.float64), B)
        w = np.repeat(w, B, axis=0) + d_l
        mut = np.maximum(np.abs(h_l - h_p), MIN_DIST)
        loss_host += float(
            (np.abs(d_l.astype(np.float64)).sum(axis=1) / mut).sum()
        )
    acc6 = w  # [262144, 16] accumulated weights at level 6

    in_maps = []
    for i in range(NCORES):
        s7 = slice(o[7] + i * LPC, o[7] + (i + 1) * LPC)
        s6 = slice(o[6] + i * N6C, o[6] + (i + 1) * N6C)
        in_maps.append(
            {
                "d7": np.ascontiguousarray(deltas[s7].T),
                "x3": np.ascontiguousarray(x[i * LPC : (i + 1) * LPC].T),
                "a6": np.ascontiguousarray(acc6[i * N6C : (i + 1) * N6C].T),
                "h6": np.ascontiguousarray(heights[s6]),
                "h7": np.ascontiguousarray(heights[s7]),
            }
        )

    nc = _get_nc()
    import os

    trace = bool(int(os.environ.get("KERNEL_TRACE", "0")))
    res = run_bass_kernel_spmd(
        nc, in_maps, core_ids=list(range(NCORES)), trace=trace
    )
    _STATE["last_results"] = res

    out = np.concatenate(
        [res.results[i]["out"].reshape(-1) for i in range(NCORES)]
    )
    loss = loss_host + sum(
        float(res.results[i]["lp"].astype(np.float64).sum())
        for i in range(NCORES)
    )
    return out, np.array([loss], dtype=np.float32)
